# revision 33
# baseline (speedup 1.0000x reference)
"""BVH Qwen router adapter kernel for 8x Trainium2 NeuronCores.

Strategy: data-parallel over tokens (2048 tokens/core), router weights
replicated. Host pre-transposes hidden_states to [H, T] so the contraction
dim lands on SBUF partitions.

Matmul: fp32 PE matmuls cost 4 cycles/row, so the fp32 GEMM is decomposed
into an exact hi/lo fp16 split running at 1 cycle/row:
    x = x_hi + x_lo/2048          (fp16 halves, residual prescaled so it
    w*256 = w_hi + w_lo/2048       stays in fp16 normal range; weights
                                   prescaled x256 to dodge denormals)
    256*logits ~= x_hi.w_hi + (x_hi.w_lo + x_lo.w_hi)/2048
The dropped x_lo.w_lo term is ~2e-7 — measured MORE accurate than a
numpy fp32 matmul (rms 9e-8 vs 2e-7 against float64). Per 128-token tile
this is two PSUM accumulation chains of N=256 fp16 passes (16 for hi.hi,
32 for the cross terms), measured at 109ns/pass — the PE streaming
roofline.

Softmax / top-32 candidate mask / top-8 select run per-token along the
free dim on DVE/ACT/GPSIMD while PE continues with the next tile. Top-k
tie semantics match jax.lax.top_k (max8/max_index/match_replace resolve
duplicates by ascending index). Final top-8 ranks by the computed
full_probs values, as the reference does. ~16 dummy matmuls on zeroed
SBUF warm the PE HAM clock-gate during the initial DMA wait.

DMA note: each dma_start costs ~650ns of serialized sync-sequencer time
regardless of size, and queue credits make every blocked trigger
head-of-line block later ones. So: x hi/lo chunk pairs ride single
[128,2,512] DMAs, weights load in ramp-up groups (1,1,2,4,8 chunks)
interleaved with the first x chunks, mid-kernel outputs are staged per
512-token super-tile and leave via the otherwise-idle GPSIMD SWDGE
queue, and the last super-tile's outputs stream per-128-token slice on
the sync queue so the kernel-tail drain waits on almost nothing.
"""

import numpy as np

TOKENS = 16384
HIDDEN = 2048
E = 128          # num experts
TOPK = 8
NCAND = 32
NCORES = 8
P = 128          # partitions
TL = TOKENS // NCORES        # tokens per core
CH = HIDDEN // P             # 16 contraction chunks
STW = 512                    # token super-tile width (one DMA per h-chunk)
WG = 4                       # weight chunks per grouped DMA
WS = 256.0                   # weight prescale
LS = 2048.0                  # lo-part prescale
NEG = -1.0e30

_nc_cache = {}


def _build(tl=TL):
    import concourse.bacc as bacc
    import concourse.mybir as mybir
    from concourse.tile import TileContext

    f16 = mybir.dt.float16
    f32 = mybir.dt.float32
    u32 = mybir.dt.uint32
    i32 = mybir.dt.int32
    X = mybir.AxisListType.X
    Exp = mybir.ActivationFunctionType.Exp

    n_st = tl // STW             # super tiles
    n_sub = STW // P             # 128-token tiles per super tile

    nc = bacc.Bacc()
    # x2: [2H, tl] = [x_hi ; x_lo*2048] fp16; wh/wl: [H, 2E] fp16 halves
    x_d = nc.dram_tensor("x2", [2 * HIDDEN, tl], f16, kind="ExternalInput")
    wh_d = nc.dram_tensor("wh", [HIDDEN, 2 * E], f16, kind="ExternalInput")
    wl_d = nc.dram_tensor("wl", [HIDDEN, 2 * E], f16, kind="ExternalInput")
    p_d = nc.dram_tensor("probs", [tl, E], f32, kind="ExternalOutput")
    v_d = nc.dram_tensor("topv", [tl, TOPK], f32, kind="ExternalOutput")
    i_d = nc.dram_tensor("topi", [tl, TOPK], i32, kind="ExternalOutput")

    with TileContext(nc) as tc:
        with (
            tc.tile_pool(name="wpool", bufs=1) as wpool,
            tc.tile_pool(name="xpool", bufs=3 * CH) as xpool,
            tc.tile_pool(name="opool", bufs=3) as opool,
            tc.tile_pool(name="spool", bufs=6) as spool,
            tc.tile_pool(name="psum", bufs=4, space="PSUM") as psum_pool,
        ):
            xr = x_d.rearrange("(z c p) t -> p z c t", p=P, z=2)
            whr = wh_d.rearrange("(c p) e -> p c e", p=P)
            wlr = wl_d.rearrange("(c p) e -> p c e", p=P)

            # Ramp-up weight groups: tiny first groups so the very first
            # matmuls only wait on ~384KB of DMA, larger groups later.
            wh_t = []        # wh_t[c] -> AP [P, 2E]
            wl_t = []
            xt0 = []         # first super-tile: (hi AP, lo AP) per chunk
            c = 0
            for g, gsz in enumerate([1, 1, 2, 4, 8]):
                wg_t = wpool.tile([P, gsz, 2 * E], f16, tag=f"w{g}", name=f"w{g}")
                nc.sync.dma_start(wg_t[:], whr[:, c:c + gsz, :])
                for q in range(gsz):
                    wh_t.append(wg_t[:, q, :])
                for q in range(gsz):
                    cc = c + q
                    if cc < 2:
                        # z-split so the first matmul waits on 128KB, not 256
                        xh = xpool.tile([P, STW], f16, tag=f"xh0{cc}",
                                        name=f"xh0_{cc}", bufs=1)
                        nc.sync.dma_start(xh[:], xr[:, 0, cc, 0:STW])
                        xl = xpool.tile([P, STW], f16, tag=f"xl0{cc}",
                                        name=f"xl0_{cc}", bufs=1)
                        nc.sync.dma_start(xl[:], xr[:, 1, cc, 0:STW])
                        xt0.append((xh[:], xl[:]))
                    else:
                        xc = xpool.tile([P, 2, STW], f16, tag="x",
                                        name=f"x_0_{cc}")
                        nc.sync.dma_start(xc[:], xr[:, :, cc, 0:STW])
                        xt0.append((xc[:, 0, :], xc[:, 1, :]))
                c += gsz
            for g in range(2):
                wlg = wpool.tile([P, CH // 2, 2 * E], f16, tag=f"wl{g}",
                                 name=f"wl{g}")
                nc.sync.dma_start(wlg[:], wlr[:, g * (CH // 2):(g + 1) * (CH // 2), :])
                for q in range(CH // 2):
                    wl_t.append(wlg[:, q, :])

            # HAM warmup: ~3.4us of dummy matmuls on zeroed SBUF while the
            # first input DMAs are in flight, so real matmuls start at the
            # un-throttled 2.4GHz PE clock.
            zdum = spool.tile([P, 2 * E], f16, tag="zdum", name="zdum", bufs=1)
            nc.gpsimd.memset(zdum[:], 0.0)
            psD = psum_pool.tile([P, 2 * E], f32, tag="psA", name="psD", bufs=4)
            for _ in range(16):
                nc.tensor.matmul(psD[:], zdum[:, 0:P], zdum[:],
                                 start=True, stop=True)

            for st in range(n_st):
                if st == 0:
                    xt = xt0
                else:
                    xt = []
                    for c in range(CH):
                        xc = xpool.tile([P, 2, STW], f16, tag="x", name=f"x_{st}_{c}")
                        nc.sync.dma_start(xc[:], xr[:, :, c, st * STW:(st + 1) * STW])
                        xt.append((xc[:, 0, :], xc[:, 1, :]))

                # per-super-tile output staging (one DMA per tensor)
                probs_o = opool.tile([P, n_sub, E], f32, tag="probs_o",
                                     name=f"probs_o{st}")
                topv_o = opool.tile([P, n_sub, TOPK], f32, tag="topv_o",
                                    name=f"topv_o{st}")
                topi_o = opool.tile([P, n_sub, TOPK], i32, tag="topi_o",
                                    name=f"topi_o{st}")

                for sub in range(n_sub):
                    t0 = st * n_sub + sub
                    cols = slice(sub * P, (sub + 1) * P)

                    # 3 fp16 chains, all N=256 (measured 109ns/pass):
                    # psA = x_hi.w_hi ; psB = x_hi.w_lo + x_lo.w_hi
                    psA = psum_pool.tile([P, 2 * E], f32, tag="psA",
                                         name=f"psA{t0}", bufs=4)
                    psB = psum_pool.tile([P, 2 * E], f32, tag="psB",
                                         name=f"psB{t0}", bufs=4)
                    for c in range(CH):
                        nc.tensor.matmul(
                            psA[:], xt[c][0][:, cols], wh_t[c],
                            start=(c == 0), stop=(c == CH - 1),
                        )
                    for c in range(CH):
                        nc.tensor.matmul(
                            psB[:], xt[c][0][:, cols], wl_t[c],
                            start=(c == 0), stop=False,
                        )
                        nc.tensor.matmul(
                            psB[:], xt[c][1][:, cols], wh_t[c],
                            start=False, stop=(c == CH - 1),
                        )

                    # combine hi and lo partials -> raw logits (x256);
                    # cols 0:E bvh, E:2E orig. The top-32 rounds later mutate
                    # the bvh half in place (exp reads the orig half first).
                    hi_s = spool.tile([P, 2 * E], f32, tag="hi_s", name=f"hi_s{t0}")
                    nc.scalar.copy(hi_s[:], psA[:])
                    raw = spool.tile([P, 2 * E], f32, tag="raw", name=f"raw{t0}")
                    nc.vector.scalar_tensor_tensor(
                        raw[:], in0=psB[:], scalar=1.0 / LS, in1=hi_s[:],
                        op0=mybir.AluOpType.mult, op1=mybir.AluOpType.add)
                    bvh = raw[:, 0:E]
                    orig = raw[:, E:2 * E]

                    # softmax over original-router logits. No max-subtraction:
                    # logits are ~N(0,1), exp stays well inside fp32 range.
                    # scale=1/WS undoes the weight prescale exactly.
                    pe_t = spool.tile([P, E], f32, tag="pe", name=f"pe{t0}")
                    s_t = spool.tile([P, 1], f32, tag="S", name=f"S{t0}")
                    nc.scalar.activation(pe_t[:], orig[:], Exp,
                                         bias=0.0, scale=1.0 / WS,
                                         accum_out=s_t[:])
                    rs_t = spool.tile([P, 1], f32, tag="rS", name=f"rS{t0}")
                    nc.vector.reciprocal(rs_t[:], s_t[:])
                    probs = probs_o[:, sub, :]
                    nc.scalar.mul(probs, pe_t[:], rs_t[:])

                    # bvh top-32 candidate mask, ranked on raw bvh logits
                    # (monotone in the reference's bvh probs); rounds mutate
                    # the bvh tile in place.
                    mx8 = spool.tile([P, 8], f32, tag="mx8", name=f"mx8{t0}")
                    for _ in range(NCAND // 8):
                        nc.vector.max(out=mx8[:], in_=bvh[:])
                        nc.vector.match_replace(out=bvh[:], in_to_replace=mx8[:],
                                                in_values=bvh[:], imm_value=NEG)
                    masked = spool.tile([P, E], f32, tag="masked", name=f"masked{t0}")
                    nc.vector.scalar_tensor_tensor(
                        masked[:], in0=bvh[:], scalar=NEG, in1=probs,
                        op0=mybir.AluOpType.is_equal,
                        op1=mybir.AluOpType.mult)

                    # top-8 among candidates by full probs
                    tv8 = spool.tile([P, 8], f32, tag="tv8", name=f"tv8{t0}")
                    nc.vector.max(out=tv8[:], in_=masked[:])
                    idx8 = spool.tile([P, 8], u32, tag="idx8", name=f"idx8{t0}")
                    nc.vector.max_index(idx8[:], tv8[:], masked[:])
                    s8 = spool.tile([P, 1], f32, tag="s8", name=f"s8{t0}")
                    nc.vector.reduce_sum(s8[:], tv8[:], axis=X)
                    rs8 = spool.tile([P, 1], f32, tag="rs8", name=f"rs8{t0}")
                    nc.vector.reciprocal(rs8[:], s8[:])
                    nc.scalar.mul(topv_o[:, sub, :], tv8[:], rs8[:])
                    nc.gpsimd.tensor_copy(topi_o[:, sub, :], idx8[:])

                # mid-kernel outputs go out via GPSIMD SWDGE so they never
                # head-of-line block the x-prefetch triggers on the sync
                # queue; the last super-tile streams per-sub-tile slices on
                # the (by then idle) sync HWDGE queue so the kernel-tail
                # drain only waits on the final 128-token slice.
                if st == n_st - 1:
                    for sub in range(n_sub):
                        rows = slice(st * STW + sub * P, st * STW + (sub + 1) * P)
                        nc.sync.dma_start(p_d[rows, :], probs_o[:, sub, :])
                        nc.sync.dma_start(v_d[rows, :], topv_o[:, sub, :])
                        nc.sync.dma_start(i_d[rows, :], topi_o[:, sub, :])
                else:
                    rows = slice(st * STW, (st + 1) * STW)
                    nc.gpsimd.dma_start(
                        p_d[rows, :].rearrange("(s p) e -> p s e", p=P),
                        probs_o[:])
                    nc.gpsimd.dma_start(
                        v_d[rows, :].rearrange("(s p) k -> p s k", p=P),
                        topv_o[:])
                    nc.gpsimd.dma_start(
                        i_d[rows, :].rearrange("(s p) k -> p s k", p=P),
                        topi_o[:])

    nc.finalize()
    return nc


def get_nc(tl=TL):
    if tl not in _nc_cache:
        _nc_cache[tl] = _build(tl)
    return _nc_cache[tl]


def _split_f16(a, scale):
    hi = a.astype(np.float16)
    lo = ((a - hi.astype(np.float32)) * scale).astype(np.float16)
    return hi, lo


def kernel(hidden_states, original_weight, bvh_weight, trace=False):
    from concourse.bass_utils import run_bass_kernel_spmd

    nc = get_nc()

    xT = np.ascontiguousarray(np.asarray(hidden_states, dtype=np.float32)).T
    x_hi, x_lo = _split_f16(xT, LS)
    x2 = np.concatenate([x_hi, x_lo], axis=0)  # [2H, T] fp16

    wcat = np.concatenate(
        [np.asarray(bvh_weight, np.float32).T,
         np.asarray(original_weight, np.float32).T], axis=1)  # [H, 2E]
    w_hi, w_lo = _split_f16(wcat * np.float32(WS), LS)
    w_hi = np.ascontiguousarray(w_hi)
    w_lo = np.ascontiguousarray(w_lo)

    in_maps = [
        {"x2": np.ascontiguousarray(x2[:, c * TL:(c + 1) * TL]),
         "wh": w_hi, "wl": w_lo}
        for c in range(NCORES)
    ]
    r = run_bass_kernel_spmd(nc, in_maps, core_ids=list(range(NCORES)),
                             trace=trace)
    res = r.results
    full_probs = np.concatenate([m["probs"] for m in res], axis=0)
    top_vals = np.concatenate([m["topv"] for m in res], axis=0)
    top_idx = np.concatenate([m["topi"] for m in res], axis=0)
    if trace:
        kernel.last_result = r
    return full_probs, top_vals, top_idx


# revision 34
# speedup vs baseline: 1.0659x; 1.0659x over previous
"""BVH Qwen router adapter kernel for 8x Trainium2 NeuronCores.

Strategy: data-parallel over tokens (2048 tokens/core), router weights
replicated. Host pre-transposes hidden_states to [H, T] so the contraction
dim lands on SBUF partitions.

Matmul: fp32 PE matmuls cost 4 cycles/row, so the fp32 GEMM is decomposed
into an exact hi/lo fp16 split running at 1 cycle/row:
    x = x_hi + x_lo/2048          (fp16 halves, residual prescaled so it
    w*256 = w_hi + w_lo/2048       stays in fp16 normal range; weights
                                   prescaled x256 to dodge denormals)
    256*logits ~= x_hi.w_hi + (x_hi.w_lo + x_lo.w_hi)/2048
The dropped x_lo.w_lo term is ~2e-7 — measured MORE accurate than a
numpy fp32 matmul (rms 9e-8 vs 2e-7 against float64). Per 128-token tile
this is two PSUM accumulation chains of N=256 fp16 passes (16 for hi.hi,
32 for the cross terms), measured at 109ns/pass — the PE streaming
roofline.

Softmax / top-32 candidate mask / top-8 select run per-token along the
free dim on DVE/ACT/GPSIMD while PE continues with the next tile. Top-k
tie semantics match jax.lax.top_k (max8/max_index/match_replace resolve
duplicates by ascending index). Final top-8 ranks by the computed
full_probs values, as the reference does. ~16 dummy matmuls on zeroed
SBUF warm the PE HAM clock-gate during the initial DMA wait.

DMA note: each dma_start costs ~650ns of serialized sync-sequencer time
regardless of size, and queue credits make every blocked trigger
head-of-line block later ones. So: x hi/lo chunk pairs ride single
[128,2,512] DMAs, weights load in ramp-up groups (1,1,2,4,8 chunks)
interleaved with the first x chunks, mid-kernel outputs are staged per
512-token super-tile and leave via the otherwise-idle GPSIMD SWDGE
queue, and the last super-tile's outputs stream per-128-token slice on
the sync queue so the kernel-tail drain waits on almost nothing.
"""

import numpy as np

TOKENS = 16384
HIDDEN = 2048
E = 128          # num experts
TOPK = 8
NCAND = 32
NCORES = 8
P = 128          # partitions
TL = TOKENS // NCORES        # tokens per core
CH = HIDDEN // P             # 16 contraction chunks
STW = 512                    # token super-tile width (one DMA per h-chunk)
WG = 4                       # weight chunks per grouped DMA
WS = 256.0                   # weight prescale
LS = 2048.0                  # lo-part prescale
NEG = -1.0e30

_nc_cache = {}


def _build(tl=TL):
    import concourse.bacc as bacc
    import concourse.mybir as mybir
    from concourse.tile import TileContext

    f16 = mybir.dt.float16
    f32 = mybir.dt.float32
    u32 = mybir.dt.uint32
    i32 = mybir.dt.int32
    X = mybir.AxisListType.X
    Exp = mybir.ActivationFunctionType.Exp

    n_st = tl // STW             # super tiles
    n_sub = STW // P             # 128-token tiles per super tile

    nc = bacc.Bacc()
    # x2: [2H, tl] = [x_hi ; x_lo*2048] fp16.  w2: [H, 4E] = [w_hi | w_lo*2048]
    x_d = nc.dram_tensor("x2", [2 * HIDDEN, tl], f16, kind="ExternalInput")
    w_d = nc.dram_tensor("w2", [HIDDEN, 4 * E], f16, kind="ExternalInput")
    p_d = nc.dram_tensor("probs", [tl, E], f32, kind="ExternalOutput")
    v_d = nc.dram_tensor("topv", [tl, TOPK], f32, kind="ExternalOutput")
    i_d = nc.dram_tensor("topi", [tl, TOPK], i32, kind="ExternalOutput")

    with TileContext(nc) as tc:
        with (
            tc.tile_pool(name="wpool", bufs=1) as wpool,
            tc.tile_pool(name="xpool", bufs=3 * CH) as xpool,
            tc.tile_pool(name="opool", bufs=3) as opool,
            tc.tile_pool(name="spool", bufs=6) as spool,
            tc.tile_pool(name="psum", bufs=4, space="PSUM") as psum_pool,
        ):
            xr = x_d.rearrange("(z c p) t -> p z c t", p=P, z=2)
            wrc = w_d.rearrange("(c p) e -> p c e", p=P)

            # Ramp-up weight groups: tiny first groups so the very first
            # matmuls only wait on ~384KB of DMA, larger groups later.
            wt = []          # wt[c] -> AP [P, 4E] (cols 0:2E hi, 2E:4E lo)
            xt0 = []         # first super-tile: (hi AP, lo AP) per chunk
            c = 0
            for g, gsz in enumerate([1, 1, 2, 4, 8]):
                wg_t = wpool.tile([P, gsz, 4 * E], f16, tag=f"w{g}", name=f"w{g}")
                nc.sync.dma_start(wg_t[:], wrc[:, c:c + gsz, :].rearrange(
                    "p c e -> p c e"))
                for q in range(gsz):
                    wt.append(wg_t[:, q, :])
                for q in range(gsz):
                    cc = c + q
                    if cc < 2:
                        # z-split so the first matmul waits on 128KB, not 256
                        xh = xpool.tile([P, STW], f16, tag=f"xh0{cc}",
                                        name=f"xh0_{cc}", bufs=1)
                        nc.sync.dma_start(xh[:], xr[:, 0, cc, 0:STW])
                        xl = xpool.tile([P, STW], f16, tag=f"xl0{cc}",
                                        name=f"xl0_{cc}", bufs=1)
                        nc.sync.dma_start(xl[:], xr[:, 1, cc, 0:STW])
                        xt0.append((xh[:], xl[:]))
                    else:
                        xc = xpool.tile([P, 2, STW], f16, tag="x",
                                        name=f"x_0_{cc}")
                        nc.sync.dma_start(xc[:], xr[:, :, cc, 0:STW])
                        xt0.append((xc[:, 0, :], xc[:, 1, :]))
                c += gsz

            # HAM warmup: ~3.4us of dummy matmuls on zeroed SBUF while the
            # first input DMAs are in flight, so real matmuls start at the
            # un-throttled 2.4GHz PE clock.
            zdum = spool.tile([P, 2 * E], f16, tag="zdum", name="zdum", bufs=1)
            nc.gpsimd.memset(zdum[:], 0.0)
            psD = psum_pool.tile([P, 2 * E], f32, tag="psA", name="psD", bufs=4)
            for _ in range(16):
                nc.tensor.matmul(psD[:], zdum[:, 0:P], zdum[:],
                                 start=True, stop=True)

            for st in range(n_st):
                if st == 0:
                    xt = xt0
                else:
                    xt = []
                    for c in range(CH):
                        xc = xpool.tile([P, 2, STW], f16, tag="x", name=f"x_{st}_{c}")
                        nc.sync.dma_start(xc[:], xr[:, :, c, st * STW:(st + 1) * STW])
                        xt.append((xc[:, 0, :], xc[:, 1, :]))

                # per-super-tile output staging (one DMA per tensor)
                probs_o = opool.tile([P, n_sub, E], f32, tag="probs_o",
                                     name=f"probs_o{st}")
                topv_o = opool.tile([P, n_sub, TOPK], f32, tag="topv_o",
                                    name=f"topv_o{st}")
                topi_o = opool.tile([P, n_sub, TOPK], i32, tag="topi_o",
                                    name=f"topi_o{st}")

                for sub in range(n_sub):
                    t0 = st * n_sub + sub
                    cols = slice(sub * P, (sub + 1) * P)

                    # 3 fp16 chains, all N=256 (measured 109ns/pass):
                    # psA = x_hi.w_hi ; psB = x_hi.w_lo + x_lo.w_hi
                    psA = psum_pool.tile([P, 2 * E], f32, tag="psA",
                                         name=f"psA{t0}", bufs=4)
                    psB = psum_pool.tile([P, 2 * E], f32, tag="psB",
                                         name=f"psB{t0}", bufs=4)
                    for c in range(CH):
                        nc.tensor.matmul(
                            psA[:], xt[c][0][:, cols], wt[c][:, 0:2 * E],
                            start=(c == 0), stop=(c == CH - 1),
                        )
                    for c in range(CH):
                        nc.tensor.matmul(
                            psB[:], xt[c][0][:, cols], wt[c][:, 2 * E:4 * E],
                            start=(c == 0), stop=False,
                        )
                        nc.tensor.matmul(
                            psB[:], xt[c][1][:, cols], wt[c][:, 0:2 * E],
                            start=False, stop=(c == CH - 1),
                        )

                    # combine hi and lo partials -> raw logits (x256);
                    # cols 0:E bvh, E:2E orig. The top-32 rounds later mutate
                    # the bvh half in place (exp reads the orig half first).
                    hi_s = spool.tile([P, 2 * E], f32, tag="hi_s", name=f"hi_s{t0}")
                    nc.scalar.copy(hi_s[:], psA[:])
                    raw = spool.tile([P, 2 * E], f32, tag="raw", name=f"raw{t0}")
                    nc.vector.scalar_tensor_tensor(
                        raw[:], in0=psB[:], scalar=1.0 / LS, in1=hi_s[:],
                        op0=mybir.AluOpType.mult, op1=mybir.AluOpType.add)
                    bvh = raw[:, 0:E]
                    orig = raw[:, E:2 * E]

                    # softmax over original-router logits. No max-subtraction:
                    # logits are ~N(0,1), exp stays well inside fp32 range.
                    # scale=1/WS undoes the weight prescale exactly.
                    pe_t = spool.tile([P, E], f32, tag="pe", name=f"pe{t0}")
                    s_t = spool.tile([P, 1], f32, tag="S", name=f"S{t0}")
                    nc.scalar.activation(pe_t[:], orig[:], Exp,
                                         bias=0.0, scale=1.0 / WS,
                                         accum_out=s_t[:])
                    rs_t = spool.tile([P, 1], f32, tag="rS", name=f"rS{t0}")
                    nc.vector.reciprocal(rs_t[:], s_t[:])
                    probs = probs_o[:, sub, :]
                    nc.scalar.mul(probs, pe_t[:], rs_t[:])

                    # bvh top-32 candidate mask, ranked on raw bvh logits
                    # (monotone in the reference's bvh probs); rounds mutate
                    # the bvh tile in place.
                    mx8 = spool.tile([P, 8], f32, tag="mx8", name=f"mx8{t0}")
                    for _ in range(NCAND // 8):
                        nc.vector.max(out=mx8[:], in_=bvh[:])
                        nc.vector.match_replace(out=bvh[:], in_to_replace=mx8[:],
                                                in_values=bvh[:], imm_value=NEG)
                    masked = spool.tile([P, E], f32, tag="masked", name=f"masked{t0}")
                    nc.vector.scalar_tensor_tensor(
                        masked[:], in0=bvh[:], scalar=NEG, in1=probs,
                        op0=mybir.AluOpType.is_equal,
                        op1=mybir.AluOpType.mult)

                    # top-8 among candidates by full probs
                    tv8 = spool.tile([P, 8], f32, tag="tv8", name=f"tv8{t0}")
                    nc.vector.max(out=tv8[:], in_=masked[:])
                    idx8 = spool.tile([P, 8], u32, tag="idx8", name=f"idx8{t0}")
                    nc.vector.max_index(idx8[:], tv8[:], masked[:])
                    s8 = spool.tile([P, 1], f32, tag="s8", name=f"s8{t0}")
                    nc.vector.reduce_sum(s8[:], tv8[:], axis=X)
                    rs8 = spool.tile([P, 1], f32, tag="rs8", name=f"rs8{t0}")
                    nc.vector.reciprocal(rs8[:], s8[:])
                    nc.scalar.mul(topv_o[:, sub, :], tv8[:], rs8[:])
                    nc.gpsimd.tensor_copy(topi_o[:, sub, :], idx8[:])

                # mid-kernel outputs go out via GPSIMD SWDGE so they never
                # head-of-line block the x-prefetch triggers on the sync
                # queue; the last super-tile streams per-sub-tile slices on
                # the (by then idle) sync HWDGE queue so the kernel-tail
                # drain only waits on the final 128-token slice.
                if st == n_st - 1:
                    for sub in range(n_sub):
                        rows = slice(st * STW + sub * P, st * STW + (sub + 1) * P)
                        nc.sync.dma_start(p_d[rows, :], probs_o[:, sub, :])
                        nc.sync.dma_start(v_d[rows, :], topv_o[:, sub, :])
                        nc.sync.dma_start(i_d[rows, :], topi_o[:, sub, :])
                else:
                    rows = slice(st * STW, (st + 1) * STW)
                    nc.gpsimd.dma_start(
                        p_d[rows, :].rearrange("(s p) e -> p s e", p=P),
                        probs_o[:])
                    nc.gpsimd.dma_start(
                        v_d[rows, :].rearrange("(s p) k -> p s k", p=P),
                        topv_o[:])
                    nc.gpsimd.dma_start(
                        i_d[rows, :].rearrange("(s p) k -> p s k", p=P),
                        topi_o[:])

    nc.finalize()
    return nc


def get_nc(tl=TL):
    if tl not in _nc_cache:
        _nc_cache[tl] = _build(tl)
    return _nc_cache[tl]


def _split_f16(a, scale):
    hi = a.astype(np.float16)
    lo = ((a - hi.astype(np.float32)) * scale).astype(np.float16)
    return hi, lo


def kernel(hidden_states, original_weight, bvh_weight, trace=False):
    from concourse.bass_utils import run_bass_kernel_spmd

    nc = get_nc()

    xT = np.ascontiguousarray(np.asarray(hidden_states, dtype=np.float32)).T
    x_hi, x_lo = _split_f16(xT, LS)
    x2 = np.concatenate([x_hi, x_lo], axis=0)  # [2H, T] fp16

    wcat = np.concatenate(
        [np.asarray(bvh_weight, np.float32).T,
         np.asarray(original_weight, np.float32).T], axis=1)  # [H, 2E]
    w_hi, w_lo = _split_f16(wcat * np.float32(WS), LS)
    w2 = np.ascontiguousarray(np.concatenate([w_hi, w_lo], axis=1))  # [H, 4E]

    in_maps = [
        {"x2": np.ascontiguousarray(x2[:, c * TL:(c + 1) * TL]), "w2": w2}
        for c in range(NCORES)
    ]
    r = run_bass_kernel_spmd(nc, in_maps, core_ids=list(range(NCORES)),
                             trace=trace)
    res = r.results
    full_probs = np.concatenate([m["probs"] for m in res], axis=0)
    top_vals = np.concatenate([m["topv"] for m in res], axis=0)
    top_idx = np.concatenate([m["topi"] for m in res], axis=0)
    if trace:
        kernel.last_result = r
    return full_probs, top_vals, top_idx


# revision 35
# speedup vs baseline: 1.0768x; 1.0102x over previous
"""BVH Qwen router adapter kernel for 8x Trainium2 NeuronCores.

Strategy: data-parallel over tokens (2048 tokens/core), router weights
replicated. Host pre-transposes hidden_states to [H, T] so the contraction
dim lands on SBUF partitions.

Matmul: fp32 PE matmuls cost 4 cycles/row, so the fp32 GEMM is decomposed
into an exact hi/lo fp16 split running at 1 cycle/row:
    x = x_hi + x_lo/2048          (fp16 halves, residual prescaled so it
    w*256 = w_hi + w_lo/2048       stays in fp16 normal range; weights
                                   prescaled x256 to dodge denormals)
    256*logits ~= x_hi.w_hi + (x_hi.w_lo + x_lo.w_hi)/2048
The dropped x_lo.w_lo term is ~2e-7 — measured MORE accurate than a
numpy fp32 matmul (rms 9e-8 vs 2e-7 against float64). Per 128-token tile
this is two PSUM accumulation chains of N=256 fp16 passes (16 for hi.hi,
32 for the cross terms), measured at 109ns/pass — the PE streaming
roofline.

Softmax / top-32 candidate mask / top-8 select run per-token along the
free dim on DVE/ACT/GPSIMD while PE continues with the next tile. Top-k
tie semantics match jax.lax.top_k (max8/max_index/match_replace resolve
duplicates by ascending index). Final top-8 ranks by the computed
full_probs values, as the reference does. ~16 dummy matmuls on zeroed
SBUF warm the PE HAM clock-gate during the initial DMA wait.

DMA note: each dma_start costs ~650ns of serialized sync-sequencer time
regardless of size, and queue credits make every blocked trigger
head-of-line block later ones. So: x hi/lo chunk pairs ride single
[128,2,512] DMAs, weights load in ramp-up groups (1,1,2,4,8 chunks)
interleaved with the first x chunks, mid-kernel outputs are staged per
512-token super-tile and leave via the otherwise-idle GPSIMD SWDGE
queue, and the last super-tile's outputs stream per-128-token slice on
the sync queue so the kernel-tail drain waits on almost nothing.
"""

import numpy as np

TOKENS = 16384
HIDDEN = 2048
E = 128          # num experts
TOPK = 8
NCAND = 32
NCORES = 8
P = 128          # partitions
TL = TOKENS // NCORES        # tokens per core
CH = HIDDEN // P             # 16 contraction chunks
STW = 512                    # token super-tile width (one DMA per h-chunk)
WG = 4                       # weight chunks per grouped DMA
WS = 256.0                   # weight prescale
LS = 2048.0                  # lo-part prescale
NEG = -1.0e30

_nc_cache = {}


def _build(tl=TL):
    import concourse.bacc as bacc
    import concourse.mybir as mybir
    from concourse.tile import TileContext

    f16 = mybir.dt.float16
    f32 = mybir.dt.float32
    u32 = mybir.dt.uint32
    i32 = mybir.dt.int32
    X = mybir.AxisListType.X
    Exp = mybir.ActivationFunctionType.Exp

    n_st = tl // STW             # super tiles
    n_sub = STW // P             # 128-token tiles per super tile

    nc = bacc.Bacc()
    # x2: per-super-tile blocked [n_st, 2H, STW] = [x_hi ; x_lo*2048] fp16,
    # so every chunk DMA is a fully contiguous read. w2: [H,4E]=[w_hi|w_lo*2048]
    x_d = nc.dram_tensor("x2", [(tl // STW) * 2 * HIDDEN, STW], f16,
                         kind="ExternalInput")
    w_d = nc.dram_tensor("w2", [HIDDEN, 4 * E], f16, kind="ExternalInput")
    p_d = nc.dram_tensor("probs", [tl, E], f32, kind="ExternalOutput")
    v_d = nc.dram_tensor("topv", [tl, TOPK], f32, kind="ExternalOutput")
    i_d = nc.dram_tensor("topi", [tl, TOPK], i32, kind="ExternalOutput")

    with TileContext(nc) as tc:
        with (
            tc.tile_pool(name="wpool", bufs=1) as wpool,
            tc.tile_pool(name="xpool", bufs=3 * CH) as xpool,
            tc.tile_pool(name="opool", bufs=3) as opool,
            tc.tile_pool(name="spool", bufs=6) as spool,
            tc.tile_pool(name="psum", bufs=4, space="PSUM") as psum_pool,
        ):
            xr = x_d.rearrange("(s z c p) t -> p s z c t", p=P, z=2, s=n_st)
            wrc = w_d.rearrange("(c p) e -> p c e", p=P)

            # Ramp-up weight groups: tiny first groups so the very first
            # matmuls only wait on ~384KB of DMA, larger groups later.
            wt = []          # wt[c] -> AP [P, 4E] (cols 0:2E hi, 2E:4E lo)
            xt0 = []         # first super-tile: (hi AP, lo AP) per chunk
            c = 0
            for g, gsz in enumerate([1, 1, 2, 4, 8]):
                wg_t = wpool.tile([P, gsz, 4 * E], f16, tag=f"w{g}", name=f"w{g}")
                nc.sync.dma_start(wg_t[:], wrc[:, c:c + gsz, :].rearrange(
                    "p c e -> p c e"))
                for q in range(gsz):
                    wt.append(wg_t[:, q, :])
                for q in range(gsz):
                    cc = c + q
                    if cc < 2:
                        # z-split so the first matmul waits on 128KB, not 256
                        xh = xpool.tile([P, STW], f16, tag=f"xh0{cc}",
                                        name=f"xh0_{cc}", bufs=1)
                        nc.sync.dma_start(xh[:], xr[:, 0, 0, cc, :])
                        xl = xpool.tile([P, STW], f16, tag=f"xl0{cc}",
                                        name=f"xl0_{cc}", bufs=1)
                        nc.sync.dma_start(xl[:], xr[:, 0, 1, cc, :])
                        xt0.append((xh[:], xl[:]))
                    else:
                        xc = xpool.tile([P, 2, STW], f16, tag="x",
                                        name=f"x_0_{cc}")
                        nc.sync.dma_start(xc[:], xr[:, 0, :, cc, :])
                        xt0.append((xc[:, 0, :], xc[:, 1, :]))
                c += gsz

            # HAM warmup: ~3.4us of dummy matmuls on zeroed SBUF while the
            # first input DMAs are in flight, so real matmuls start at the
            # un-throttled 2.4GHz PE clock.
            zdum = spool.tile([P, 2 * E], f16, tag="zdum", name="zdum", bufs=1)
            nc.gpsimd.memset(zdum[:], 0.0)
            psD = psum_pool.tile([P, 2 * E], f32, tag="psA", name="psD", bufs=4)
            for _ in range(16):
                nc.tensor.matmul(psD[:], zdum[:, 0:P], zdum[:],
                                 start=True, stop=True)

            for st in range(n_st):
                if st == 0:
                    xt = xt0
                else:
                    xt = []
                    for c in range(CH):
                        xc = xpool.tile([P, 2, STW], f16, tag="x", name=f"x_{st}_{c}")
                        nc.sync.dma_start(xc[:], xr[:, st, :, c, :])
                        xt.append((xc[:, 0, :], xc[:, 1, :]))

                # per-super-tile output staging (one DMA per tensor)
                probs_o = opool.tile([P, n_sub, E], f32, tag="probs_o",
                                     name=f"probs_o{st}")
                topv_o = opool.tile([P, n_sub, TOPK], f32, tag="topv_o",
                                    name=f"topv_o{st}")
                topi_o = opool.tile([P, n_sub, TOPK], i32, tag="topi_o",
                                    name=f"topi_o{st}")

                for sub in range(n_sub):
                    t0 = st * n_sub + sub
                    cols = slice(sub * P, (sub + 1) * P)

                    # 3 fp16 chains, all N=256 (measured 109ns/pass):
                    # psA = x_hi.w_hi ; psB = x_hi.w_lo + x_lo.w_hi
                    psA = psum_pool.tile([P, 2 * E], f32, tag="psA",
                                         name=f"psA{t0}", bufs=4)
                    psB = psum_pool.tile([P, 2 * E], f32, tag="psB",
                                         name=f"psB{t0}", bufs=4)
                    for c in range(CH):
                        nc.tensor.matmul(
                            psA[:], xt[c][0][:, cols], wt[c][:, 0:2 * E],
                            start=(c == 0), stop=(c == CH - 1),
                        )
                    for c in range(CH):
                        nc.tensor.matmul(
                            psB[:], xt[c][0][:, cols], wt[c][:, 2 * E:4 * E],
                            start=(c == 0), stop=False,
                        )
                        nc.tensor.matmul(
                            psB[:], xt[c][1][:, cols], wt[c][:, 0:2 * E],
                            start=False, stop=(c == CH - 1),
                        )

                    # combine hi and lo partials -> raw logits (x256);
                    # cols 0:E bvh, E:2E orig. The top-32 rounds later mutate
                    # the bvh half in place (exp reads the orig half first).
                    hi_s = spool.tile([P, 2 * E], f32, tag="hi_s", name=f"hi_s{t0}")
                    nc.scalar.copy(hi_s[:], psA[:])
                    raw = spool.tile([P, 2 * E], f32, tag="raw", name=f"raw{t0}")
                    nc.vector.scalar_tensor_tensor(
                        raw[:], in0=psB[:], scalar=1.0 / LS, in1=hi_s[:],
                        op0=mybir.AluOpType.mult, op1=mybir.AluOpType.add)
                    bvh = raw[:, 0:E]
                    orig = raw[:, E:2 * E]

                    # softmax over original-router logits. No max-subtraction:
                    # logits are ~N(0,1), exp stays well inside fp32 range.
                    # scale=1/WS undoes the weight prescale exactly.
                    pe_t = spool.tile([P, E], f32, tag="pe", name=f"pe{t0}")
                    s_t = spool.tile([P, 1], f32, tag="S", name=f"S{t0}")
                    nc.scalar.activation(pe_t[:], orig[:], Exp,
                                         bias=0.0, scale=1.0 / WS,
                                         accum_out=s_t[:])
                    rs_t = spool.tile([P, 1], f32, tag="rS", name=f"rS{t0}")
                    nc.vector.reciprocal(rs_t[:], s_t[:])
                    probs = probs_o[:, sub, :]
                    nc.scalar.mul(probs, pe_t[:], rs_t[:])

                    # bvh top-32 candidate mask, ranked on raw bvh logits
                    # (monotone in the reference's bvh probs); rounds mutate
                    # the bvh tile in place.
                    mx8 = spool.tile([P, 8], f32, tag="mx8", name=f"mx8{t0}")
                    for _ in range(NCAND // 8):
                        nc.vector.max(out=mx8[:], in_=bvh[:])
                        nc.vector.match_replace(out=bvh[:], in_to_replace=mx8[:],
                                                in_values=bvh[:], imm_value=NEG)
                    masked = spool.tile([P, E], f32, tag="masked", name=f"masked{t0}")
                    nc.vector.scalar_tensor_tensor(
                        masked[:], in0=bvh[:], scalar=NEG, in1=probs,
                        op0=mybir.AluOpType.is_equal,
                        op1=mybir.AluOpType.mult)

                    # top-8 among candidates by full probs
                    tv8 = spool.tile([P, 8], f32, tag="tv8", name=f"tv8{t0}")
                    nc.vector.max(out=tv8[:], in_=masked[:])
                    idx8 = spool.tile([P, 8], u32, tag="idx8", name=f"idx8{t0}")
                    nc.vector.max_index(idx8[:], tv8[:], masked[:])
                    s8 = spool.tile([P, 1], f32, tag="s8", name=f"s8{t0}")
                    nc.vector.reduce_sum(s8[:], tv8[:], axis=X)
                    rs8 = spool.tile([P, 1], f32, tag="rs8", name=f"rs8{t0}")
                    nc.vector.reciprocal(rs8[:], s8[:])
                    nc.scalar.mul(topv_o[:, sub, :], tv8[:], rs8[:])
                    nc.gpsimd.tensor_copy(topi_o[:, sub, :], idx8[:])

                # mid-kernel outputs go out via GPSIMD SWDGE so they never
                # head-of-line block the x-prefetch triggers on the sync
                # queue; the last super-tile streams per-sub-tile slices on
                # the (by then idle) sync HWDGE queue so the kernel-tail
                # drain only waits on the final 128-token slice.
                if st == n_st - 1:
                    for sub in range(n_sub):
                        rows = slice(st * STW + sub * P, st * STW + (sub + 1) * P)
                        nc.sync.dma_start(p_d[rows, :], probs_o[:, sub, :])
                        nc.sync.dma_start(v_d[rows, :], topv_o[:, sub, :])
                        nc.sync.dma_start(i_d[rows, :], topi_o[:, sub, :])
                else:
                    rows = slice(st * STW, (st + 1) * STW)
                    nc.gpsimd.dma_start(
                        p_d[rows, :].rearrange("(s p) e -> p s e", p=P),
                        probs_o[:])
                    nc.gpsimd.dma_start(
                        v_d[rows, :].rearrange("(s p) k -> p s k", p=P),
                        topv_o[:])
                    nc.gpsimd.dma_start(
                        i_d[rows, :].rearrange("(s p) k -> p s k", p=P),
                        topi_o[:])

    nc.finalize()
    return nc


def get_nc(tl=TL):
    if tl not in _nc_cache:
        _nc_cache[tl] = _build(tl)
    return _nc_cache[tl]


def _split_f16(a, scale):
    hi = a.astype(np.float16)
    lo = ((a - hi.astype(np.float32)) * scale).astype(np.float16)
    return hi, lo


def kernel(hidden_states, original_weight, bvh_weight, trace=False):
    from concourse.bass_utils import run_bass_kernel_spmd

    nc = get_nc()

    xT = np.ascontiguousarray(np.asarray(hidden_states, dtype=np.float32)).T
    x_hi, x_lo = _split_f16(xT, LS)
    x2 = np.concatenate([x_hi, x_lo], axis=0)  # [2H, T] fp16

    wcat = np.concatenate(
        [np.asarray(bvh_weight, np.float32).T,
         np.asarray(original_weight, np.float32).T], axis=1)  # [H, 2E]
    w_hi, w_lo = _split_f16(wcat * np.float32(WS), LS)
    w2 = np.ascontiguousarray(np.concatenate([w_hi, w_lo], axis=1))  # [H, 4E]

    n_st = TL // STW

    def _blocked(xc):
        # [2H, TL] -> [n_st*2H, STW] with super-tile-major contiguous blocks
        return np.ascontiguousarray(
            xc.reshape(2 * HIDDEN, n_st, STW).transpose(1, 0, 2)
        ).reshape(n_st * 2 * HIDDEN, STW)

    in_maps = [
        {"x2": _blocked(x2[:, c * TL:(c + 1) * TL]), "w2": w2}
        for c in range(NCORES)
    ]
    r = run_bass_kernel_spmd(nc, in_maps, core_ids=list(range(NCORES)),
                             trace=trace)
    res = r.results
    full_probs = np.concatenate([m["probs"] for m in res], axis=0)
    top_vals = np.concatenate([m["topv"] for m in res], axis=0)
    top_idx = np.concatenate([m["topi"] for m in res], axis=0)
    if trace:
        kernel.last_result = r
    return full_probs, top_vals, top_idx
